# revision 38
# baseline (speedup 1.0000x reference)
"""Trainium2 Bass kernel for Bahdanau-style additive attention.

    h_proj = hidden @ W_attn[:H] + b_attn                # (B, H)
    e_proj = encoder_outputs @ W_attn[H:]                # (B, S, H)
    energy = tanh(h_proj[:, None, :] + e_proj)           # (B, S, H)
    att    = energy @ v                                  # (B, S)
    out    = softmax(att, axis=1)                        # (B, S)

B=32, S=2048, H=1024. Data-parallel over batch: 4 batches per core on 8
NeuronCores. Per-core kernel (all matmul inputs bf16, fp32 accumulation):

  - encoder rows stream in via SWDGE cast-DMA (fp32->bf16), then one xbar
    SBUF->SBUF DMA transpose per 512-row chunk puts H on partitions:
    xt[p, r, k, j] = enc[r*128+j, k*128+p], so the PE can contract over H.
  - e_proj^T tiles [h_out=128, s=512] accumulate over 8 k-tiles in PSUM,
    with We tiles stationary (native [h_in, h_out] layout, no transpose).
  - ScalarE fuses the h_proj bias add + tanh in one pass (bias is
    per-partition in this layout), writing bf16 to SBUF.
  - The v-dot is 8 M=1 matmuls (lhsT = v^T column) issued adjacently as two
    column-tiled groups of 4 (tile_position=(0, 32j)) so they run
    concurrently in the PE array; the 4 partial rows land on PSUM
    partitions {0,32,64,96} and are reduced by ScalarE copies + VectorE
    adds (one PSUM operand per instruction).
  - Softmax: ScalarE exp with fused accum_out partial sums, final
    reciprocal + scale on one partition per batch.

Startup pipelining (the preamble used to serialize ~28us of PE idle):

  - Weights stream as 16 per-m-block fp32 DMAs on the two HWDGE queues
    (SP gets We + the chunk transposes, ACT gets Wh), each cast to bf16
    by the DVE through a 4-deep fp32 staging pool. Block m unblocks
    m-group m of chunk 0, so the PE starts after ~1 block + 1 chunk.
  - chunk (0,0) loads as 4 row-band DMAs + 4 xbar transposes so the
    first transpose overlaps the later bands' HBM reads.
  - hidden^T / b^T / v^T are produced by small padded xbar transposes
    (16-partition staging tiles) instead of 4-byte gather DMAs.
  - h_proj m-tiles (tiny matmuls) interleave with chunk 0's m-groups on
    the PE queue; tanh[m] only needs h_proj[m].
  - The last chunk's v-dot runs as 8 sequential accumulating M=1 matmuls
    into one PSUM row (no column tiling), so the tail is just
    exp-from-PSUM -> reciprocal -> scale -> DMA.
"""
import numpy as np

B, S, H = 32, 2048, 1024
N_CORES = 8
B_LOCAL = B // N_CORES          # 4 batches per core
SL = B_LOCAL * S                # 8192 encoder rows per core
KT = H // 128                   # 8 contraction tiles
MT = H // 128                   # 8 output-H tiles
S_CHUNK = 512
RT = S_CHUNK // 128             # 4 row sub-tiles per chunk
N_CHUNKS = S // S_CHUNK         # 4 chunks per batch

_CACHE = {}


def _build(num_devices=N_CORES, reps=1):
    import concourse.mybir as mybir
    import concourse.tile as tile
    from concourse import bacc

    f32 = mybir.dt.float32

    nc = bacc.Bacc("TRN2", target_bir_lowering=False, debug=False,
                   num_devices=num_devices)
    enc = nc.dram_tensor("enc", [SL, H], f32, kind="ExternalInput").ap()
    hidden = nc.dram_tensor("hidden", [B_LOCAL, H], f32, kind="ExternalInput").ap()
    w_attn = nc.dram_tensor("w_attn", [2 * H, H], f32, kind="ExternalInput").ap()
    b_attn = nc.dram_tensor("b_attn", [H], f32, kind="ExternalInput").ap()
    v_in = nc.dram_tensor("v", [H], f32, kind="ExternalInput").ap()
    out = nc.dram_tensor("out", [B_LOCAL, S], f32, kind="ExternalOutput").ap()

    with tile.TileContext(nc) as tc:
        _emit(nc, tc, enc, hidden, w_attn, b_attn, v_in, out, reps=reps)

    nc.compile()
    return nc


def _emit(nc, tc, enc, hidden, w_attn, b_attn, v_in, out, reps=1):
    import concourse.mybir as mybir

    f32 = mybir.dt.float32
    bf16 = mybir.dt.bfloat16
    with (
        tc.tile_pool(name="weights", bufs=1) as w_pool,
        tc.tile_pool(name="small", bufs=1) as small_pool,
        tc.tile_pool(name="raw", bufs=4) as raw_pool,
        tc.tile_pool(name="xt", bufs=4) as xt_pool,
        tc.tile_pool(name="tanh", bufs=16) as tanh_pool,
        tc.tile_pool(name="perbatch", bufs=3) as pb_pool,
        tc.tile_pool(name="psum_e", bufs=6, space="PSUM") as psum_e_pool,
        tc.tile_pool(name="psum_l", bufs=1, space="PSUM") as psum_l_pool,
        tc.tile_pool(name="psum_hp", bufs=1, space="PSUM") as psum_hp_pool,
    ):
        # All preamble loads ride the one SWDGE (Pool) queue in priority
        # order — its desc-gen pipeline issues transfers in exactly this
        # order, which the global DMA fabric then respects:
        #   chunk0 bands -> We m-blocks (cast bf16) -> chunks 1/2 -> smalls
        #   -> Wh. Transposes ride the otherwise-empty SP HWDGE queue.
        # we_sb/wh_sb layout [p, k, h_out]: h_in = k*128 + p (native rows).
        we_sb = w_pool.tile([128, KT, H], bf16)
        wh_sb = w_pool.tile([128, KT, H], bf16)

        # ---- chunk (0,0) raw load (4 row bands), then We halves (cast-DMA
        # to bf16). Chunk 0 contracts in two k-half passes, so pass A only
        # needs half A of We + this chunk's transpose. ----
        raw0 = raw_pool.tile([128, RT, H], bf16, tag="raw")
        nc.gpsimd.dma_start(
            out=raw0[:],
            in_=enc[0:S_CHUNK, :].rearrange("(r j) h -> j r h", j=128))
        nc.gpsimd.dma_start(
            out=we_sb[:, :4], in_=w_attn[H:H + 512].rearrange(
                "(k p) h -> p k h", p=128))
        nc.gpsimd.dma_start(
            out=we_sb[:, 4:], in_=w_attn[H + 512:].rearrange(
                "(k p) h -> p k h", p=128))
        # ---- chunks (0,1) and (0,2) raw prefetch ----
        raw1 = raw_pool.tile([128, RT, H], bf16, tag="raw")
        nc.gpsimd.dma_start(
            out=raw1[:],
            in_=enc[S_CHUNK:2 * S_CHUNK, :].rearrange("(r j) h -> j r h", j=128))
        raw2 = raw_pool.tile([128, RT, H], bf16, tag="raw")
        nc.gpsimd.dma_start(
            out=raw2[:],
            in_=enc[2 * S_CHUNK:3 * S_CHUNK, :].rearrange(
                "(r j) h -> j r h", j=128))
        # ---- small pads (needed only ~chunk 2) ----
        hidp = small_pool.tile([16, H], bf16)
        btp = small_pool.tile([16, 128], bf16)
        vtp = small_pool.tile([16, 128], bf16)
        nc.vector.memset(hidp[:], 0)
        nc.vector.memset(btp[:], 0)
        nc.vector.memset(vtp[:], 0)
        nc.gpsimd.dma_start(out=hidp[0:B_LOCAL, :], in_=hidden)
        nc.gpsimd.dma_start(out=btp[0:KT, :],
                            in_=b_attn.rearrange("(t p) -> t p", p=128))
        nc.gpsimd.dma_start(out=vtp[0:KT, :],
                            in_=v_in.rearrange("(t p) -> t p", p=128))
        # ---- Wh: big cast-DMAs, last. Nothing waits on them early: chunk
        # 0/1 tanh is deferred until h_proj exists (see _main_loop). ----
        nc.gpsimd.dma_start(
            out=wh_sb[:, :4], in_=w_attn[0:512].rearrange(
                "(k p) h -> p k h", p=128))
        nc.gpsimd.dma_start(
            out=wh_sb[:, 4:], in_=w_attn[512:H].rearrange(
                "(k p) h -> p k h", p=128))

        # ---- transposes on the SP HWDGE queue (nothing else there) ----
        xt0 = xt_pool.tile([128, RT, KT, 128], bf16, tag="xt")
        nc.sync.dma_start_transpose(xt0[:], raw0[:])
        xt1 = xt_pool.tile([128, RT, KT, 128], bf16, tag="xt")
        nc.sync.dma_start_transpose(xt1[:], raw1[:])
        xt2 = xt_pool.tile([128, RT, KT, 128], bf16, tag="xt")
        nc.sync.dma_start_transpose(xt2[:], raw2[:])
        # ---- small transposes (first read ~chunk 2) ----
        # ht[p, k, j] = hidp[j, k*128+p]; valid j < B_LOCAL
        ht = small_pool.tile([128, KT, 16], bf16)
        nc.sync.dma_start_transpose(ht[:], hidp[:])
        btT = small_pool.tile([128, 16], bf16)   # btT[p, t] = b_attn[t*128+p]
        nc.sync.dma_start_transpose(btT[:], btp[:])
        btT32 = small_pool.tile([128, 16], f32)
        nc.vector.tensor_scalar_add(out=btT32[:], in0=btT[:], scalar1=0.0)
        vtT = small_pool.tile([128, 16], bf16)   # vtT[p, t] = v[t*128+p]
        nc.sync.dma_start_transpose(vtT[:], vtp[:])

        hp_sb = small_pool.tile([128, MT, B_LOCAL], f32)

        pre = {"wh_sb": wh_sb, "ht": ht, "btT": btT32, "hp_sb": hp_sb,
               "psum_hp": psum_hp_pool}

        # ---- main loop over (batch, s-chunk) ----
        for _rep in range(reps):
            _main_loop(nc, tc, mybir, enc, out, raw_pool, xt_pool, tanh_pool,
                       pb_pool, psum_e_pool, psum_l_pool, we_sb, vtT, hp_sb,
                       prefetched=({0: xt0, 1: xt1, 2: xt2}
                                   if _rep == 0 else None),
                       pre=(pre if _rep == 0 else None))


def _load_chunk(nc, mybir, enc, base, raw_pool, xt_pool):
    """Load 512 encoder rows (cast fp32->bf16), then one whole-chunk xbar
    transpose puts H on partitions:
      raw[j, r, h] = enc[base + r*128 + j, h]
      xt[p, r, k, j] = raw[j, r, k*128 + p]   (128-column block transpose)
    so xt[:, :, k, :] is a (strided) [128, 512] rhs tile."""
    bf16 = mybir.dt.bfloat16
    raw = raw_pool.tile([128, RT, H], bf16, tag="raw")
    nc.gpsimd.dma_start(
        out=raw[:],
        in_=enc[base:base + S_CHUNK, :].rearrange("(r j) h -> j r h", j=128))
    xt = xt_pool.tile([128, RT, KT, 128], bf16, tag="xt")
    nc.sync.dma_start_transpose(xt[:], raw[:])
    return xt


def _main_loop(nc, tc, mybir, enc, out, raw_pool, xt_pool, tanh_pool, pb_pool,
               psum_e_pool, psum_l_pool, we_sb, vt_sb, hp_sb, prefetched=None,
               pre=None):
    """Software-pipelined chunk loop. The second v-dot group of chunk i
    depends on that chunk's last tanh (ScalarE), so it is deferred into
    chunk i+1's PE stream (after its m=0 matmuls) — by then the tanh has
    long finished and the PE never stalls on ScalarE.

    When `pre` is given (first rep), chunk (0,0) additionally interleaves
    the weight-block casts and the h_proj m-tiles so the whole weight
    preamble overlaps the first chunk's compute."""
    f32 = mybir.dt.float32
    bf16 = mybir.dt.bfloat16

    def flush_pending(p):
        """Emit deferred v-dot group B + row-reduce + exp (+ batch tail)."""
        psl4 = p["psl4"]
        for j in range(4):
            m = j + 4
            nc.tensor.matmul(psl4[32 * j:32 * j + 1, :],
                             vt_sb[:, m:m + 1], p["ths"][m][:],
                             start=False, stop=True,
                             tile_position=(0, 32 * j))
        # reduce the 4 rows (ACT copies + DVE adds; 1 PSUM input max)
        s0 = pb_pool.tile([1, S_CHUNK], f32, tag="s0", bufs=2)
        s1 = pb_pool.tile([1, S_CHUNK], f32, tag="s1", bufs=2)
        t0 = pb_pool.tile([1, S_CHUNK], f32, tag="t0", bufs=2)
        t1 = pb_pool.tile([1, S_CHUNK], f32, tag="t1", bufs=2)
        od = pb_pool.tile([1, S_CHUNK], f32, tag="od", bufs=2)
        nc.scalar.copy(out=s0[:], in_=psl4[0:1, :])
        nc.vector.tensor_add(t0[:], psl4[32:33, :], s0[:])
        nc.scalar.copy(out=s1[:], in_=psl4[64:65, :])
        nc.vector.tensor_add(t1[:], psl4[96:97, :], s1[:])
        nc.vector.tensor_add(od[:], t0[:], t1[:])
        c, ex_sb, ssum = p["c"], p["ex_sb"], p["ssum"]
        nc.scalar.activation(
            out=ex_sb[0:1, c * S_CHUNK:(c + 1) * S_CHUNK],
            in_=od[:], func=mybir.ActivationFunctionType.Exp,
            accum_out=ssum[0:1, c:c + 1])
        if c == N_CHUNKS - 1:
            _batch_tail(p["b"], ex_sb, ssum)

    def _batch_tail(bb, ex_sb, ssum):
        # softmax normalize for the finished batch
        # (no max-subtraction needed: |logit| <= ~26)
        sm = pb_pool.tile([1, 2], f32, tag="sm")
        nc.vector.reduce_sum(out=sm[0:1, 0:1], in_=ssum[0:1, 0:N_CHUNKS],
                             axis=mybir.AxisListType.X)
        nc.vector.reciprocal(out=sm[0:1, 1:2], in_=sm[0:1, 0:1])
        ot = pb_pool.tile([1, S], f32, tag="ot", bufs=1)
        nc.vector.tensor_scalar_mul(out=ot[:], in0=ex_sb[:],
                                    scalar1=sm[0:1, 1:2])
        nc.sync.dma_start(out=out[bb:bb + 1, :], in_=ot[:])

    pending = None
    defer = None          # chunks (0,0)/(0,1) stored pre-tanh (first rep)
    dvd = None            # their deferred v-dot state during chunk (0,2)
    ex_sb = ssum = None
    for b in range(B_LOCAL):
        ex_sb = pb_pool.tile([1, S], f32, tag="ex", bufs=2)
        ssum = pb_pool.tile([1, N_CHUNKS + 2], f32, tag="ssum")
        for c in range(N_CHUNKS):
            base = b * S + c * S_CHUNK
            last = (b == B_LOCAL - 1 and c == N_CHUNKS - 1)
            if b == 0 and prefetched is not None and c in prefetched:
                xt = prefetched[c]
            else:
                xt = _load_chunk(nc, mybir, enc, base, raw_pool, xt_pool)

            if pre is not None and b == 0 and c < 2:
                # --- deferred mode: h_proj isn't ready yet (Wh is still
                # streaming). Store raw pre-tanh e_proj tiles; tanh + v-dot
                # happen at chunk (0,2) once hp_sb exists. ---
                if defer is None:
                    defer = []
                if c == 0:
                    # two k-half passes across all 8 PSUM banks: pass A
                    # starts as soon as We half A + the transpose land
                    pses = []
                    for m in range(MT):
                        if m < 6:
                            pse = psum_e_pool.tile([128, S_CHUNK], f32)
                        elif m == 6:
                            pse = psum_l_pool.tile([128, S_CHUNK], f32,
                                                   tag="psl")
                        else:
                            pse = pre["psum_hp"].tile([128, S_CHUNK], f32,
                                                      tag="pshp")
                        pses.append(pse)
                    for m in range(MT):
                        for k in range(KT // 2):
                            nc.tensor.matmul(
                                pses[m][:],
                                we_sb[:, k, m * 128:(m + 1) * 128],
                                xt[:, :, k, :],
                                start=(k == 0), stop=False)
                    for m in range(MT):
                        for k in range(KT // 2, KT):
                            nc.tensor.matmul(
                                pses[m][:],
                                we_sb[:, k, m * 128:(m + 1) * 128],
                                xt[:, :, k, :],
                                start=False, stop=(k == KT - 1))
                        est = tanh_pool.tile([128, S_CHUNK], bf16, tag="est")
                        nc.scalar.copy(out=est[:], in_=pses[m][:])
                        defer.append(est)
                else:
                    for m in range(MT):
                        pse = psum_e_pool.tile([128, S_CHUNK], f32)
                        for k in range(KT):
                            nc.tensor.matmul(
                                pse[:],
                                we_sb[:, k, m * 128:(m + 1) * 128],
                                xt[:, :, k, :],
                                start=(k == 0), stop=(k == KT - 1))
                        est = tanh_pool.tile([128, S_CHUNK], bf16, tag="est")
                        nc.scalar.copy(out=est[:], in_=pse[:])
                        defer.append(est)
                continue

            if pre is not None and b == 0 and c == 2:
                # --- h_proj: 8 tiny m-tiles + bias adds (Wh landed during
                # chunks 0/1), then tanh the 16 stored tiles ---
                ps_hp = pre["psum_hp"].tile([128, MT * B_LOCAL], f32,
                                            tag="pshp")
                for m in range(MT):
                    for k in range(KT):
                        nc.tensor.matmul(
                            ps_hp[:, m * B_LOCAL:(m + 1) * B_LOCAL],
                            pre["wh_sb"][:, k, m * 128:(m + 1) * 128],
                            pre["ht"][:, k, 0:B_LOCAL],
                            start=(k == 0), stop=(k == KT - 1))
                for m in range(MT):
                    nc.vector.tensor_scalar_add(
                        out=hp_sb[:, m, :],
                        in0=ps_hp[:, m * B_LOCAL:(m + 1) * B_LOCAL],
                        scalar1=pre["btT"][:, m:m + 1])
                dths = []
                for i, est in enumerate(defer):
                    m = i % MT
                    dth = tanh_pool.tile([128, S_CHUNK], bf16)
                    nc.scalar.activation(
                        out=dth[:], in_=est[:],
                        func=mybir.ActivationFunctionType.Tanh,
                        bias=hp_sb[:, m, 0:1], scale=1.0)
                    dths.append(dth)
                # one PSUM bank, rows 0 / 32 for the two chunks' v-dots
                psl_d = pre["psum_hp"].tile([33, S_CHUNK], f32, tag="pshp")
                dvd = {"dths": dths, "psl_d": psl_d, "next": 0}
                defer = None

            psl4 = None
            psl_seq = None
            ths = []
            for m in range(MT):
                pse = psum_e_pool.tile([128, S_CHUNK], f32)
                for k in range(KT):
                    nc.tensor.matmul(pse[:],
                                     we_sb[:, k, m * 128:(m + 1) * 128],
                                     xt[:, :, k, :],
                                     start=(k == 0), stop=(k == KT - 1))
                if last and m >= 1:
                    # sequential v-dot for m-1 (accumulate into one row)
                    nc.tensor.matmul(psl_seq[0:1, :],
                                     vt_sb[:, m - 1:m], ths[m - 1][:],
                                     start=(m == 1), stop=False)
                if dvd is not None and m >= 1:
                    # two deferred v-dots per m-group slot (16 total); the
                    # pair targets col groups 0/32 so it runs concurrently
                    for _ in range(2):
                        i = dvd["next"]
                        if i < 2 * MT:
                            mm, cc = divmod(i, 2)
                            nc.tensor.matmul(
                                dvd["psl_d"][32 * cc:32 * cc + 1, :],
                                vt_sb[:, mm:mm + 1],
                                dvd["dths"][cc * MT + mm][:],
                                start=(mm == 0), stop=(mm == MT - 1),
                                tile_position=(0, 32 * cc))
                            dvd["next"] = i + 1
                th = tanh_pool.tile([128, S_CHUNK], bf16)
                nc.scalar.activation(
                    out=th[:], in_=pse[:],
                    func=mybir.ActivationFunctionType.Tanh,
                    bias=hp_sb[:, m, b:b + 1], scale=1.0)
                ths.append(th)
                if m == 0:
                    if pending is not None:
                        flush_pending(pending)
                        pending = None
                    if last:
                        psl_seq = psum_l_pool.tile([128, S_CHUNK], f32,
                                                   tag="psl")
                if m == 4 and not last:
                    # v-dot group A (m=0..3): column-tiled, concurrent in
                    # the PE array; tanh m<=3 finished during the m=4 MMs
                    psl4 = psum_l_pool.tile([128, S_CHUNK], f32, tag="psl")
                    for j in range(4):
                        nc.tensor.matmul(psl4[32 * j:32 * j + 1, :],
                                         vt_sb[:, j:j + 1], ths[j][:],
                                         start=True, stop=False,
                                         tile_position=(0, 32 * j))
            if dvd is not None:
                # drain any remaining deferred v-dots, then their exps
                while dvd["next"] < 2 * MT:
                    i = dvd["next"]
                    mm, cc = divmod(i, 2)
                    nc.tensor.matmul(
                        dvd["psl_d"][32 * cc:32 * cc + 1, :],
                        vt_sb[:, mm:mm + 1],
                        dvd["dths"][cc * MT + mm][:],
                        start=(mm == 0), stop=(mm == MT - 1),
                        tile_position=(0, 32 * cc))
                    dvd["next"] = i + 1
                for cc in range(2):
                    nc.scalar.activation(
                        out=ex_sb[0:1, cc * S_CHUNK:(cc + 1) * S_CHUNK],
                        in_=dvd["psl_d"][32 * cc:32 * cc + 1, :],
                        func=mybir.ActivationFunctionType.Exp,
                        accum_out=ssum[0:1, cc:cc + 1])
                dvd = None
            if last:
                # final v-dot + tail: exp straight from PSUM, then normalize
                nc.tensor.matmul(psl_seq[0:1, :], vt_sb[:, MT - 1:MT],
                                 ths[MT - 1][:], start=False, stop=True)
                nc.scalar.activation(
                    out=ex_sb[0:1, c * S_CHUNK:(c + 1) * S_CHUNK],
                    in_=psl_seq[0:1, :],
                    func=mybir.ActivationFunctionType.Exp,
                    accum_out=ssum[0:1, c:c + 1])
                _batch_tail(b, ex_sb, ssum)
            else:
                pending = {"psl4": psl4, "ths": ths, "b": b, "c": c,
                           "ex_sb": ex_sb, "ssum": ssum}


def _build_runner():
    """Compile once and build a persistent jitted SPMD executor."""
    import jax
    from jax.sharding import Mesh, PartitionSpec
    from jax.experimental.shard_map import shard_map
    import concourse.mybir as mybir
    from concourse import bass2jax

    nc = _build()
    bass2jax.install_neuronx_cc_hook()

    partition_name = nc.partition_id_tensor.name if nc.partition_id_tensor else None
    in_names, out_names, out_avals, zero_outs = [], [], [], []
    for alloc in nc.m.functions[0].allocations:
        if not isinstance(alloc, mybir.MemoryLocationSet):
            continue
        name = alloc.memorylocations[0].name
        if alloc.kind == "ExternalInput":
            if name != partition_name:
                in_names.append(name)
        elif alloc.kind == "ExternalOutput":
            out_names.append(name)
            shape = tuple(alloc.tensor_shape)
            dtype = mybir.dt.np(alloc.dtype)
            out_avals.append(jax.core.ShapedArray(shape, dtype))
            zero_outs.append(np.zeros(shape, dtype))
    n_params = len(in_names)
    n_outs = len(out_avals)
    in_names = list(in_names) + list(out_names)
    if partition_name is not None:
        in_names.append(partition_name)
    donate = tuple(range(n_params, n_params + n_outs))

    def _body(*args):
        operands = list(args)
        if partition_name is not None:
            operands.append(bass2jax.partition_id_tensor())
        outs = bass2jax._bass_exec_p.bind(
            *operands,
            out_avals=tuple(out_avals),
            in_names=tuple(in_names),
            out_names=tuple(out_names),
            lowering_input_output_aliases=(),
            sim_require_finite=True,
            sim_require_nnan=True,
            nc=nc,
        )
        return tuple(outs)

    devices = jax.devices()[:N_CORES]
    assert len(devices) >= N_CORES, f"need {N_CORES} devices"
    mesh = Mesh(np.asarray(devices[:N_CORES]), ("core",))
    in_specs = (PartitionSpec("core"),) * (n_params + n_outs)
    out_specs = (PartitionSpec("core"),) * len(out_names)
    sharded = jax.jit(
        shard_map(_body, mesh=mesh, in_specs=in_specs, out_specs=out_specs,
                  check_rep=False),
        donate_argnums=donate, keep_unused=True)
    sharding = jax.sharding.NamedSharding(mesh, PartitionSpec("core"))

    state = {
        "sharded": sharded,
        "sharding": sharding,
        "in_names": in_names[:n_params],
        "out_names": out_names,
        "out_avals": out_avals,
        "zero_outs": zero_outs,
        "jax": jax,
    }
    return state


def _get_state():
    if "state" not in _CACHE:
        _CACHE["state"] = _build_runner()
    return _CACHE["state"]


def prepare_in_maps(hidden, encoder_outputs, W_attn, b_attn, v):
    """Shard inputs: batch-split encoder_outputs, replicate the rest."""
    enc = np.ascontiguousarray(np.asarray(encoder_outputs, dtype=np.float32))
    hid = np.ascontiguousarray(np.asarray(hidden, dtype=np.float32))
    W = np.ascontiguousarray(np.asarray(W_attn, dtype=np.float32))
    bb = np.ascontiguousarray(np.asarray(b_attn, dtype=np.float32))
    vv = np.ascontiguousarray(np.asarray(v, dtype=np.float32))
    in_maps = []
    for c in range(N_CORES):
        shard = enc[c * B_LOCAL:(c + 1) * B_LOCAL].reshape(SL, H)
        hshard = hid[c * B_LOCAL:(c + 1) * B_LOCAL]
        in_maps.append({"enc": shard, "hidden": hshard, "w_attn": W,
                        "b_attn": bb, "v": vv})
    return in_maps


def device_inputs(in_maps):
    st = _get_state()
    jax = st["jax"]
    concat_in = [
        np.concatenate([np.asarray(m[name]) for m in in_maps], axis=0)
        for name in st["in_names"]
    ]
    dev = [jax.device_put(a, st["sharding"]) for a in concat_in]
    jax.block_until_ready(dev)
    return dev


def run_device(dev_in):
    """One SPMD execution; returns the (B, S) fp32 output."""
    st = _get_state()
    jax = st["jax"]
    zeros = [
        jax.device_put(np.zeros((N_CORES * z.shape[0], *z.shape[1:]), z.dtype),
                       st["sharding"])
        for z in st["zero_outs"]
    ]
    out_arrs = st["sharded"](*dev_in, *zeros)
    jax.block_until_ready(out_arrs)
    i = st["out_names"].index("out")
    full = np.asarray(out_arrs[i]).reshape(N_CORES, B_LOCAL, S)
    return full.reshape(B, S)


def kernel(hidden, encoder_outputs, W_attn, b_attn, v):
    in_maps = prepare_in_maps(hidden, encoder_outputs, W_attn, b_attn, v)
    dev_in = device_inputs(in_maps)
    return run_device(dev_in).astype(np.float32)


# revision 45
# speedup vs baseline: 1.0271x; 1.0271x over previous
"""Trainium2 Bass kernel for Bahdanau-style additive attention.

    h_proj = hidden @ W_attn[:H] + b_attn                # (B, H)
    e_proj = encoder_outputs @ W_attn[H:]                # (B, S, H)
    energy = tanh(h_proj[:, None, :] + e_proj)           # (B, S, H)
    att    = energy @ v                                  # (B, S)
    out    = softmax(att, axis=1)                        # (B, S)

B=32, S=2048, H=1024. Data-parallel over batch: 4 batches per core on 8
NeuronCores. Per-core kernel (fp32 PSUM accumulation throughout):

  - encoder rows stream in via SWDGE cast-DMA (fp32->bf16), then one xbar
    SBUF->SBUF DMA transpose per 512-row chunk puts H on partitions:
    xt[p, r, k, j] = enc[r*128+j, k*128+p], so the PE can contract over H.
  - e_proj^T tiles [h_out=128, s=512] accumulate in PSUM with We tiles
    stationary (native [h_in, h_out] layout). The first 256 h_in contract
    as ONE fp8e4 DoubleRow matmul (2 fp8 weights/PE cell, ~1.44x bf16
    throughput); the remaining 6 k-tiles stay bf16. All of We is scaled
    by 64 (exact in bf16) so the fp8 operand clears e4m3's subnormal
    floor; tanh descales via its activation scale=1/64. Measured output
    rel err 0.0156 vs the fp32 reference (gate 2e-2); predicted exactly
    by an ml_dtypes numpy simulation of the quantization.
  - the fp8 moving tile xt8[p, pair, r, j] is cast per chunk by the DVE
    from the bf16 transpose (pair dim outermost so the lowered DoubleRow
    ifmap AP is 3D).
  - ScalarE fuses the h_proj bias add + 1/64 descale + tanh in one pass,
    writing bf16; the v-dot is column-tiled M=1 matmuls as two concurrent
    groups of 4; ScalarE exp uses accum_out for fused softmax partials.
  - chunks prefetch 4 deep; chunk transposes ride the SP HWDGE queue.

Startup/teardown pipelining (preamble+tail used to idle the PE ~38us):

  - chunk (0,0) + We halves load first; chunk 0 contracts in two k-half
    passes across all 8 PSUM banks so pass A starts after half A of We.
  - chunk (0,0)/(0,1) run "deferred": raw pre-tanh e_proj is stored bf16
    and tanh'd at chunk (0,2), so nothing waits on Wh/h_proj early; Wh
    streams from inside the main loop. h_proj m-tiles + the 16 deferred
    tanh/v-dots interleave into chunk (0,2)'s stream.
  - hidden^T/b^T/v^T come from small padded xbar transposes on the ACT
    HWDGE queue instead of 4-byte gather DMAs.
  - ~40 throwaway matmuls on zeros warm the PE p-state during the
    preamble DMAs.
  - the last chunk's v-dot accumulates into one PSUM row (sequential
    M=1 matmuls interleaved with the m-groups), so the tail is just
    exp-from-PSUM -> reciprocal -> scale -> DMA.
"""
import numpy as np

B, S, H = 32, 2048, 1024
N_CORES = 8
B_LOCAL = B // N_CORES          # 4 batches per core
SL = B_LOCAL * S                # 8192 encoder rows per core
KT = H // 128                   # 8 contraction tiles
MT = H // 128                   # 8 output-H tiles
S_CHUNK = 512
RT = S_CHUNK // 128             # 4 row sub-tiles per chunk
N_CHUNKS = S // S_CHUNK         # 4 chunks per batch

_CACHE = {}


def _build(num_devices=N_CORES, reps=1):
    import concourse.mybir as mybir
    import concourse.tile as tile
    from concourse import bacc

    f32 = mybir.dt.float32

    nc = bacc.Bacc("TRN2", target_bir_lowering=False, debug=False,
                   num_devices=num_devices)
    enc = nc.dram_tensor("enc", [SL, H], f32, kind="ExternalInput").ap()
    hidden = nc.dram_tensor("hidden", [B_LOCAL, H], f32, kind="ExternalInput").ap()
    w_attn = nc.dram_tensor("w_attn", [2 * H, H], f32, kind="ExternalInput").ap()
    b_attn = nc.dram_tensor("b_attn", [H], f32, kind="ExternalInput").ap()
    v_in = nc.dram_tensor("v", [H], f32, kind="ExternalInput").ap()
    out = nc.dram_tensor("out", [B_LOCAL, S], f32, kind="ExternalOutput").ap()

    with tile.TileContext(nc) as tc:
        _emit(nc, tc, enc, hidden, w_attn, b_attn, v_in, out, reps=reps)

    nc.compile()
    return nc


def _emit(nc, tc, enc, hidden, w_attn, b_attn, v_in, out, reps=1):
    import concourse.mybir as mybir

    f32 = mybir.dt.float32
    bf16 = mybir.dt.bfloat16
    with (
        tc.tile_pool(name="weights", bufs=1) as w_pool,
        tc.tile_pool(name="small", bufs=1) as small_pool,
        tc.tile_pool(name="raw", bufs=4) as raw_pool,
        tc.tile_pool(name="xt", bufs=5) as xt_pool,
        tc.tile_pool(name="tanh", bufs=16) as tanh_pool,
        tc.tile_pool(name="xt8", bufs=4) as xt8_pool,
        tc.tile_pool(name="perbatch", bufs=3) as pb_pool,
        tc.tile_pool(name="psum_e", bufs=6, space="PSUM") as psum_e_pool,
        tc.tile_pool(name="psum_l", bufs=1, space="PSUM") as psum_l_pool,
        tc.tile_pool(name="psum_hp", bufs=1, space="PSUM") as psum_hp_pool,
    ):
        # All preamble loads ride the one SWDGE (Pool) queue in priority
        # order — its desc-gen pipeline issues transfers in exactly this
        # order, which the global DMA fabric then respects:
        #   chunk0 bands -> We m-blocks (cast bf16) -> chunks 1/2 -> smalls
        #   -> Wh. Transposes ride the otherwise-empty SP HWDGE queue.
        # we_sb/wh_sb layout [p, k, h_out]: h_in = k*128 + p (native rows).
        we_sb = w_pool.tile([128, KT, H], bf16)
        wh_sb = w_pool.tile([128, KT, H], bf16)

        # ---- chunk (0,0) raw load (4 row bands), then We halves (cast-DMA
        # to bf16). Chunk 0 contracts in two k-half passes, so pass A only
        # needs half A of We + this chunk's transpose. ----
        raw0 = raw_pool.tile([128, RT, H], bf16, tag="raw")
        nc.gpsimd.dma_start(
            out=raw0[:],
            in_=enc[0:S_CHUNK, :].rearrange("(r j) h -> j r h", j=128))
        nc.gpsimd.dma_start(
            out=we_sb[:, :4], in_=w_attn[H:H + 512].rearrange(
                "(k p) h -> p k h", p=128))
        nc.gpsimd.dma_start(
            out=we_sb[:, 4:], in_=w_attn[H + 512:].rearrange(
                "(k p) h -> p k h", p=128))
        # ---- small pads: tiny, loaded right behind We so their ACT-queue
        # transposes land long before chunk 2 needs them ----
        hidp = small_pool.tile([16, H], bf16)
        btp = small_pool.tile([16, 128], bf16)
        vtp = small_pool.tile([16, 128], bf16)
        nc.vector.memset(hidp[:], 0)
        nc.vector.memset(btp[:], 0)
        nc.vector.memset(vtp[:], 0)
        nc.gpsimd.dma_start(out=hidp[0:B_LOCAL, :], in_=hidden)
        nc.gpsimd.dma_start(out=btp[0:KT, :],
                            in_=b_attn.rearrange("(t p) -> t p", p=128))
        nc.gpsimd.dma_start(out=vtp[0:KT, :],
                            in_=v_in.rearrange("(t p) -> t p", p=128))
        # ---- scale We by 64 in place (exact in bf16; tanh descales by
        # 1/64) so the k0-1 fp8 pair tile is well-conditioned in e4m3;
        # per-(half, m-block) so pass A m-group m only waits half A ----
        for half in range(2):
            ks = slice(0, 4) if half == 0 else slice(4, KT)
            for m in range(MT):
                nc.vector.tensor_scalar_mul(
                    out=we_sb[:, ks, m * 128:(m + 1) * 128],
                    in0=we_sb[:, ks, m * 128:(m + 1) * 128], scalar1=64.0)
        # fp8 pair tile for the DoubleRow matmul over h_in 0:256
        we8 = w_pool.tile([128, 2, H], mybir.dt.float8e4)
        nc.vector.tensor_scalar_add(out=we8[:], in0=we_sb[:, 0:2, :],
                                    scalar1=0.0)
        # ---- chunks (0,1) and (0,2) raw prefetch ----
        raw1 = raw_pool.tile([128, RT, H], bf16, tag="raw")
        nc.gpsimd.dma_start(
            out=raw1[:],
            in_=enc[S_CHUNK:2 * S_CHUNK, :].rearrange("(r j) h -> j r h", j=128))
        raw2 = raw_pool.tile([128, RT, H], bf16, tag="raw")
        nc.gpsimd.dma_start(
            out=raw2[:],
            in_=enc[2 * S_CHUNK:3 * S_CHUNK, :].rearrange(
                "(r j) h -> j r h", j=128))

        # ---- transposes on the SP HWDGE queue (nothing else there) ----
        xt0 = xt_pool.tile([128, RT, KT, 128], bf16, tag="xt")
        nc.sync.dma_start_transpose(xt0[:], raw0[:])
        xt1 = xt_pool.tile([128, RT, KT, 128], bf16, tag="xt")
        nc.sync.dma_start_transpose(xt1[:], raw1[:])
        xt2 = xt_pool.tile([128, RT, KT, 128], bf16, tag="xt")
        nc.sync.dma_start_transpose(xt2[:], raw2[:])
        # ---- small transposes on the ACT HWDGE queue (its compute is
        # idle until chunk 0's PSUM copies) ----
        # ht[p, k, j] = hidp[j, k*128+p]; valid j < B_LOCAL
        ht = small_pool.tile([128, KT, 16], bf16)
        nc.scalar.dma_start_transpose(ht[:], hidp[:])
        btT = small_pool.tile([128, 16], bf16)   # btT[p, t] = b_attn[t*128+p]
        nc.scalar.dma_start_transpose(btT[:], btp[:])
        btT32 = small_pool.tile([128, 16], f32)
        nc.vector.tensor_scalar_add(out=btT32[:], in0=btT[:], scalar1=0.0)
        vtT = small_pool.tile([128, 16], bf16)   # vtT[p, t] = v[t*128+p]
        nc.scalar.dma_start_transpose(vtT[:], vtp[:])

        hp_sb = small_pool.tile([128, MT, B_LOCAL], f32)

        # ---- PE warm-up: throwaway matmuls on zeros while the chunk-0
        # DMAs land, so the real stream starts at full p-state ----
        warm = small_pool.tile([128, S_CHUNK], bf16)
        nc.vector.memset(warm[:], 0)
        ps_w = psum_l_pool.tile([128, S_CHUNK], f32, tag="psl")
        for _ in range(40):
            nc.tensor.matmul(ps_w[:], warm[:, 0:128], warm[:],
                             start=True, stop=True)

        pre = {"wh_sb": wh_sb, "ht": ht, "btT": btT32, "hp_sb": hp_sb,
               "psum_hp": psum_hp_pool, "w_attn": w_attn}
        fp8 = {"we8": we8, "xt8_pool": xt8_pool}

        # ---- main loop over (batch, s-chunk) ----
        for _rep in range(reps):
            _main_loop(nc, tc, mybir, enc, out, raw_pool, xt_pool, tanh_pool,
                       pb_pool, psum_e_pool, psum_l_pool, we_sb, vtT, hp_sb,
                       prefetched=({0: xt0, 1: xt1, 2: xt2}
                                   if _rep == 0 else None),
                       pre=(pre if _rep == 0 else None), fp8=fp8)


def _load_chunk(nc, mybir, enc, base, raw_pool, xt_pool):
    """Load 512 encoder rows (cast fp32->bf16), then one whole-chunk xbar
    transpose puts H on partitions:
      raw[j, r, h] = enc[base + r*128 + j, h]
      xt[p, r, k, j] = raw[j, r, k*128 + p]   (128-column block transpose)
    so xt[:, :, k, :] is a (strided) [128, 512] rhs tile."""
    bf16 = mybir.dt.bfloat16
    raw = raw_pool.tile([128, RT, H], bf16, tag="raw")
    nc.gpsimd.dma_start(
        out=raw[:],
        in_=enc[base:base + S_CHUNK, :].rearrange("(r j) h -> j r h", j=128))
    xt = xt_pool.tile([128, RT, KT, 128], bf16, tag="xt")
    nc.sync.dma_start_transpose(xt[:], raw[:])
    return xt


def _main_loop(nc, tc, mybir, enc, out, raw_pool, xt_pool, tanh_pool, pb_pool,
               psum_e_pool, psum_l_pool, we_sb, vt_sb, hp_sb, prefetched=None,
               pre=None, fp8=None):
    """Software-pipelined chunk loop. The second v-dot group of chunk i
    depends on that chunk's last tanh (ScalarE), so it is deferred into
    chunk i+1's PE stream (after its m=0 matmuls) — by then the tanh has
    long finished and the PE never stalls on ScalarE.

    When `pre` is given (first rep), chunk (0,0) additionally interleaves
    the weight-block casts and the h_proj m-tiles so the whole weight
    preamble overlaps the first chunk's compute."""
    f32 = mybir.dt.float32
    bf16 = mybir.dt.bfloat16

    def flush_pending(p):
        """Emit deferred v-dot group B + row-reduce + exp (+ batch tail)."""
        psl4 = p["psl4"]
        for j in range(4):
            m = j + 4
            nc.tensor.matmul(psl4[32 * j:32 * j + 1, :],
                             vt_sb[:, m:m + 1], p["ths"][m][:],
                             start=False, stop=True,
                             tile_position=(0, 32 * j))
        # reduce the 4 rows (ACT copies + DVE adds; 1 PSUM input max)
        s0 = pb_pool.tile([1, S_CHUNK], f32, tag="s0", bufs=2)
        s1 = pb_pool.tile([1, S_CHUNK], f32, tag="s1", bufs=2)
        t0 = pb_pool.tile([1, S_CHUNK], f32, tag="t0", bufs=2)
        t1 = pb_pool.tile([1, S_CHUNK], f32, tag="t1", bufs=2)
        od = pb_pool.tile([1, S_CHUNK], f32, tag="od", bufs=2)
        nc.scalar.copy(out=s0[:], in_=psl4[0:1, :])
        nc.vector.tensor_add(t0[:], psl4[32:33, :], s0[:])
        nc.scalar.copy(out=s1[:], in_=psl4[64:65, :])
        nc.vector.tensor_add(t1[:], psl4[96:97, :], s1[:])
        nc.vector.tensor_add(od[:], t0[:], t1[:])
        c, ex_sb, ssum = p["c"], p["ex_sb"], p["ssum"]
        nc.scalar.activation(
            out=ex_sb[0:1, c * S_CHUNK:(c + 1) * S_CHUNK],
            in_=od[:], func=mybir.ActivationFunctionType.Exp,
            accum_out=ssum[0:1, c:c + 1])
        if c == N_CHUNKS - 1:
            _batch_tail(p["b"], ex_sb, ssum)

    def _batch_tail(bb, ex_sb, ssum):
        # softmax normalize for the finished batch
        # (no max-subtraction needed: |logit| <= ~26)
        sm = pb_pool.tile([1, 2], f32, tag="sm")
        nc.vector.reduce_sum(out=sm[0:1, 0:1], in_=ssum[0:1, 0:N_CHUNKS],
                             axis=mybir.AxisListType.X)
        nc.vector.reciprocal(out=sm[0:1, 1:2], in_=sm[0:1, 0:1])
        ot = pb_pool.tile([1, S], f32, tag="ot", bufs=1)
        nc.vector.tensor_scalar_mul(out=ot[:], in0=ex_sb[:],
                                    scalar1=sm[0:1, 1:2])
        nc.sync.dma_start(out=out[bb:bb + 1, :], in_=ot[:])

    pending = None
    defer = None          # chunks (0,0)/(0,1) stored pre-tanh (first rep)
    dvd = None            # their deferred v-dot state during chunk (0,2)
    ex_sb = ssum = None
    # explicit 3-deep chunk prefetch so raw-DMA/transpose chains pipeline
    # instead of lockstepping with the PE
    n_seq = B_LOCAL * N_CHUNKS
    PF = 4
    xts = {}
    for i in range(PF):
        if prefetched is not None and i in prefetched:
            xts[i] = prefetched[i]
        else:
            xts[i] = _load_chunk(nc, mybir, enc, i * S_CHUNK, raw_pool,
                                 xt_pool)
    for b in range(B_LOCAL):
        ex_sb = pb_pool.tile([1, S], f32, tag="ex", bufs=2)
        ssum = pb_pool.tile([1, N_CHUNKS + 2], f32, tag="ssum")
        for c in range(N_CHUNKS):
            base = b * S + c * S_CHUNK
            last = (b == B_LOCAL - 1 and c == N_CHUNKS - 1)
            i_seq = b * N_CHUNKS + c
            if i_seq + PF < n_seq:
                xts[i_seq + PF] = _load_chunk(
                    nc, mybir, enc, (i_seq + PF) * S_CHUNK, raw_pool,
                    xt_pool)
            if pre is not None and i_seq == 0:
                # Wh loads here: after the early chunk raws, before h_proj
                # needs it at chunk (0,2)
                H_ = H
                nc.gpsimd.dma_start(
                    out=pre["wh_sb"][:, :4],
                    in_=pre["w_attn"][0:512].rearrange(
                        "(k p) h -> p k h", p=128))
                nc.gpsimd.dma_start(
                    out=pre["wh_sb"][:, 4:],
                    in_=pre["w_attn"][512:H_].rearrange(
                        "(k p) h -> p k h", p=128))
            xt = xts.pop(i_seq)
            # fp8 pair tile of this chunk's k-tiles 0..1 (pair dim
            # outermost so the lowered DoubleRow ifmap AP is 3D)
            xt8 = fp8["xt8_pool"].tile([128, 2, RT, 128],
                                       mybir.dt.float8e4, tag="xt8")
            nc.vector.tensor_scalar_add(
                out=xt8[:],
                in0=xt[:, :, 0:2, :].rearrange("p r k j -> p k r j"),
                scalar1=0.0)

            if pre is not None and b == 0 and c < 2:
                # --- deferred mode: h_proj isn't ready yet (Wh is still
                # streaming). Store raw pre-tanh e_proj tiles; tanh + v-dot
                # happen at chunk (0,2) once hp_sb exists. ---
                if defer is None:
                    defer = []
                if c == 0:
                    # two k-half passes across all 8 PSUM banks: pass A
                    # starts as soon as We half A + the transpose land
                    pses = []
                    for m in range(MT):
                        if m < 6:
                            pse = psum_e_pool.tile([128, S_CHUNK], f32)
                        elif m == 6:
                            pse = psum_l_pool.tile([128, S_CHUNK], f32,
                                                   tag="psl")
                        else:
                            pse = pre["psum_hp"].tile([128, S_CHUNK], f32,
                                                      tag="pshp")
                        pses.append(pse)
                    for m in range(MT):
                        for k in range(2, KT // 2):
                            nc.tensor.matmul(
                                pses[m][:],
                                we_sb[:, k, m * 128:(m + 1) * 128],
                                xt[:, :, k, :],
                                start=(k == 2), stop=False)
                        nc.tensor.matmul(
                            pses[m][:],
                            fp8["we8"][:, :, m * 128:(m + 1) * 128],
                            xt8[:],
                            start=False, stop=False,
                            perf_mode=mybir.MatmulPerfMode.DoubleRow)
                    for m in range(MT):
                        for k in range(KT // 2, KT):
                            nc.tensor.matmul(
                                pses[m][:],
                                we_sb[:, k, m * 128:(m + 1) * 128],
                                xt[:, :, k, :],
                                start=False, stop=(k == KT - 1))
                        est = tanh_pool.tile([128, S_CHUNK], bf16, tag="est")
                        nc.scalar.copy(out=est[:], in_=pses[m][:])
                        defer.append(est)
                else:
                    for m in range(MT):
                        pse = psum_e_pool.tile([128, S_CHUNK], f32)
                        for k in range(2, KT):
                            nc.tensor.matmul(
                                pse[:],
                                we_sb[:, k, m * 128:(m + 1) * 128],
                                xt[:, :, k, :],
                                start=(k == 2), stop=False)
                        nc.tensor.matmul(
                            pse[:],
                            fp8["we8"][:, :, m * 128:(m + 1) * 128],
                            xt8[:],
                            start=False, stop=True,
                            perf_mode=mybir.MatmulPerfMode.DoubleRow)
                        est = tanh_pool.tile([128, S_CHUNK], bf16, tag="est")
                        nc.scalar.copy(out=est[:], in_=pse[:])
                        defer.append(est)
                continue

            if pre is not None and b == 0 and c == 2:
                # --- h_proj: 8 tiny m-tiles + bias adds (Wh landed during
                # chunks 0/1), then tanh the 16 stored tiles ---
                ps_hp = pre["psum_hp"].tile([128, MT * B_LOCAL], f32,
                                            tag="pshp")
                for m in range(MT):
                    for k in range(KT):
                        nc.tensor.matmul(
                            ps_hp[:, m * B_LOCAL:(m + 1) * B_LOCAL],
                            pre["wh_sb"][:, k, m * 128:(m + 1) * 128],
                            pre["ht"][:, k, 0:B_LOCAL],
                            start=(k == 0), stop=(k == KT - 1))
                for m in range(MT):
                    nc.vector.tensor_scalar_add(
                        out=hp_sb[:, m, :],
                        in0=ps_hp[:, m * B_LOCAL:(m + 1) * B_LOCAL],
                        scalar1=pre["btT"][:, m:m + 1])
                dths = []
                for i, est in enumerate(defer):
                    m = i % MT
                    dth = tanh_pool.tile([128, S_CHUNK], bf16)
                    nc.scalar.activation(
                        out=dth[:], in_=est[:],
                        func=mybir.ActivationFunctionType.Tanh,
                        bias=hp_sb[:, m, 0:1], scale=1.0 / 64.0)
                    dths.append(dth)
                # one PSUM bank, rows 0 / 32 for the two chunks' v-dots
                psl_d = pre["psum_hp"].tile([33, S_CHUNK], f32, tag="pshp")
                dvd = {"dths": dths, "psl_d": psl_d, "next": 0}
                defer = None

            psl4 = None
            psl_seq = None
            ths = []
            for m in range(MT):
                pse = psum_e_pool.tile([128, S_CHUNK], f32)
                for k in range(2, KT):
                    nc.tensor.matmul(pse[:],
                                     we_sb[:, k, m * 128:(m + 1) * 128],
                                     xt[:, :, k, :],
                                     start=(k == 2), stop=False)
                nc.tensor.matmul(pse[:],
                                 fp8["we8"][:, :, m * 128:(m + 1) * 128],
                                 xt8[:],
                                 start=False, stop=True,
                                 perf_mode=mybir.MatmulPerfMode.DoubleRow)
                if last and m >= 1:
                    # sequential v-dot for m-1 (accumulate into one row)
                    nc.tensor.matmul(psl_seq[0:1, :],
                                     vt_sb[:, m - 1:m], ths[m - 1][:],
                                     start=(m == 1), stop=False)
                if dvd is not None and m >= 1:
                    # two deferred v-dots per m-group slot (16 total); the
                    # pair targets col groups 0/32 so it runs concurrently
                    for _ in range(2):
                        i = dvd["next"]
                        if i < 2 * MT:
                            mm, cc = divmod(i, 2)
                            nc.tensor.matmul(
                                dvd["psl_d"][32 * cc:32 * cc + 1, :],
                                vt_sb[:, mm:mm + 1],
                                dvd["dths"][cc * MT + mm][:],
                                start=(mm == 0), stop=(mm == MT - 1),
                                tile_position=(0, 32 * cc))
                            dvd["next"] = i + 1
                th = tanh_pool.tile([128, S_CHUNK], bf16)
                nc.scalar.activation(
                    out=th[:], in_=pse[:],
                    func=mybir.ActivationFunctionType.Tanh,
                    bias=hp_sb[:, m, b:b + 1], scale=1.0 / 64.0)
                ths.append(th)
                if m == 0:
                    if pending is not None:
                        flush_pending(pending)
                        pending = None
                    if last:
                        psl_seq = psum_l_pool.tile([128, S_CHUNK], f32,
                                                   tag="psl")
                if m == 4 and not last:
                    # v-dot group A (m=0..3): column-tiled, concurrent in
                    # the PE array; tanh m<=3 finished during the m=4 MMs
                    psl4 = psum_l_pool.tile([128, S_CHUNK], f32, tag="psl")
                    for j in range(4):
                        nc.tensor.matmul(psl4[32 * j:32 * j + 1, :],
                                         vt_sb[:, j:j + 1], ths[j][:],
                                         start=True, stop=False,
                                         tile_position=(0, 32 * j))
            if dvd is not None:
                # drain any remaining deferred v-dots, then their exps
                while dvd["next"] < 2 * MT:
                    i = dvd["next"]
                    mm, cc = divmod(i, 2)
                    nc.tensor.matmul(
                        dvd["psl_d"][32 * cc:32 * cc + 1, :],
                        vt_sb[:, mm:mm + 1],
                        dvd["dths"][cc * MT + mm][:],
                        start=(mm == 0), stop=(mm == MT - 1),
                        tile_position=(0, 32 * cc))
                    dvd["next"] = i + 1
                for cc in range(2):
                    nc.scalar.activation(
                        out=ex_sb[0:1, cc * S_CHUNK:(cc + 1) * S_CHUNK],
                        in_=dvd["psl_d"][32 * cc:32 * cc + 1, :],
                        func=mybir.ActivationFunctionType.Exp,
                        accum_out=ssum[0:1, cc:cc + 1])
                dvd = None
            if last:
                # final v-dot + tail: exp straight from PSUM, then normalize
                nc.tensor.matmul(psl_seq[0:1, :], vt_sb[:, MT - 1:MT],
                                 ths[MT - 1][:], start=False, stop=True)
                nc.scalar.activation(
                    out=ex_sb[0:1, c * S_CHUNK:(c + 1) * S_CHUNK],
                    in_=psl_seq[0:1, :],
                    func=mybir.ActivationFunctionType.Exp,
                    accum_out=ssum[0:1, c:c + 1])
                _batch_tail(b, ex_sb, ssum)
            else:
                pending = {"psl4": psl4, "ths": ths, "b": b, "c": c,
                           "ex_sb": ex_sb, "ssum": ssum}


def _build_runner():
    """Compile once and build a persistent jitted SPMD executor."""
    import jax
    from jax.sharding import Mesh, PartitionSpec
    from jax.experimental.shard_map import shard_map
    import concourse.mybir as mybir
    from concourse import bass2jax

    nc = _build()
    bass2jax.install_neuronx_cc_hook()

    partition_name = nc.partition_id_tensor.name if nc.partition_id_tensor else None
    in_names, out_names, out_avals, zero_outs = [], [], [], []
    for alloc in nc.m.functions[0].allocations:
        if not isinstance(alloc, mybir.MemoryLocationSet):
            continue
        name = alloc.memorylocations[0].name
        if alloc.kind == "ExternalInput":
            if name != partition_name:
                in_names.append(name)
        elif alloc.kind == "ExternalOutput":
            out_names.append(name)
            shape = tuple(alloc.tensor_shape)
            dtype = mybir.dt.np(alloc.dtype)
            out_avals.append(jax.core.ShapedArray(shape, dtype))
            zero_outs.append(np.zeros(shape, dtype))
    n_params = len(in_names)
    n_outs = len(out_avals)
    in_names = list(in_names) + list(out_names)
    if partition_name is not None:
        in_names.append(partition_name)
    donate = tuple(range(n_params, n_params + n_outs))

    def _body(*args):
        operands = list(args)
        if partition_name is not None:
            operands.append(bass2jax.partition_id_tensor())
        outs = bass2jax._bass_exec_p.bind(
            *operands,
            out_avals=tuple(out_avals),
            in_names=tuple(in_names),
            out_names=tuple(out_names),
            lowering_input_output_aliases=(),
            sim_require_finite=True,
            sim_require_nnan=True,
            nc=nc,
        )
        return tuple(outs)

    devices = jax.devices()[:N_CORES]
    assert len(devices) >= N_CORES, f"need {N_CORES} devices"
    mesh = Mesh(np.asarray(devices[:N_CORES]), ("core",))
    in_specs = (PartitionSpec("core"),) * (n_params + n_outs)
    out_specs = (PartitionSpec("core"),) * len(out_names)
    sharded = jax.jit(
        shard_map(_body, mesh=mesh, in_specs=in_specs, out_specs=out_specs,
                  check_rep=False),
        donate_argnums=donate, keep_unused=True)
    sharding = jax.sharding.NamedSharding(mesh, PartitionSpec("core"))

    state = {
        "sharded": sharded,
        "sharding": sharding,
        "in_names": in_names[:n_params],
        "out_names": out_names,
        "out_avals": out_avals,
        "zero_outs": zero_outs,
        "jax": jax,
    }
    return state


def _get_state():
    if "state" not in _CACHE:
        _CACHE["state"] = _build_runner()
    return _CACHE["state"]


def prepare_in_maps(hidden, encoder_outputs, W_attn, b_attn, v):
    """Shard inputs: batch-split encoder_outputs, replicate the rest."""
    enc = np.ascontiguousarray(np.asarray(encoder_outputs, dtype=np.float32))
    hid = np.ascontiguousarray(np.asarray(hidden, dtype=np.float32))
    W = np.ascontiguousarray(np.asarray(W_attn, dtype=np.float32))
    bb = np.ascontiguousarray(np.asarray(b_attn, dtype=np.float32))
    vv = np.ascontiguousarray(np.asarray(v, dtype=np.float32))
    in_maps = []
    for c in range(N_CORES):
        shard = enc[c * B_LOCAL:(c + 1) * B_LOCAL].reshape(SL, H)
        hshard = hid[c * B_LOCAL:(c + 1) * B_LOCAL]
        in_maps.append({"enc": shard, "hidden": hshard, "w_attn": W,
                        "b_attn": bb, "v": vv})
    return in_maps


def device_inputs(in_maps):
    st = _get_state()
    jax = st["jax"]
    concat_in = [
        np.concatenate([np.asarray(m[name]) for m in in_maps], axis=0)
        for name in st["in_names"]
    ]
    dev = [jax.device_put(a, st["sharding"]) for a in concat_in]
    jax.block_until_ready(dev)
    return dev


def run_device(dev_in):
    """One SPMD execution; returns the (B, S) fp32 output."""
    st = _get_state()
    jax = st["jax"]
    zeros = [
        jax.device_put(np.zeros((N_CORES * z.shape[0], *z.shape[1:]), z.dtype),
                       st["sharding"])
        for z in st["zero_outs"]
    ]
    out_arrs = st["sharded"](*dev_in, *zeros)
    jax.block_until_ready(out_arrs)
    i = st["out_names"].index("out")
    full = np.asarray(out_arrs[i]).reshape(N_CORES, B_LOCAL, S)
    return full.reshape(B, S)


def kernel(hidden, encoder_outputs, W_attn, b_attn, v):
    in_maps = prepare_in_maps(hidden, encoder_outputs, W_attn, b_attn, v)
    dev_in = device_inputs(in_maps)
    return run_device(dev_in).astype(np.float32)


# revision 46
# speedup vs baseline: 1.1714x; 1.1405x over previous
"""Trainium2 Bass kernel for Bahdanau-style additive attention.

    h_proj = hidden @ W_attn[:H] + b_attn                # (B, H)
    e_proj = encoder_outputs @ W_attn[H:]                # (B, S, H)
    energy = tanh(h_proj[:, None, :] + e_proj)           # (B, S, H)
    att    = energy @ v                                  # (B, S)
    out    = softmax(att, axis=1)                        # (B, S)

B=32, S=2048, H=1024. Data-parallel over batch: 4 batches per core on 8
NeuronCores. Per-core kernel (fp32 PSUM accumulation throughout):

  - encoder rows stream in via SWDGE cast-DMA (fp32->bf16), then one xbar
    SBUF->SBUF DMA transpose per 512-row chunk puts H on partitions:
    xt[p, r, k, j] = enc[r*128+j, k*128+p], so the PE can contract over H.
  - e_proj^T tiles [h_out=128, s=512] accumulate in PSUM with We tiles
    stationary (native [h_in, h_out] layout). The first 256 h_in contract
    as ONE fp8e4 DoubleRow matmul (2 fp8 weights/PE cell, ~1.44x bf16
    throughput); the remaining 6 k-tiles stay bf16. All of We is scaled
    by 64 (exact in bf16) so the fp8 operand clears e4m3's subnormal
    floor; tanh descales via its activation scale=1/64. Measured output
    rel err 0.0156 vs the fp32 reference (gate 2e-2); predicted exactly
    by an ml_dtypes numpy simulation of the quantization.
  - the fp8 moving tile xt8[p, pair, r, j] is cast per chunk by the DVE
    from the bf16 transpose (pair dim outermost so the lowered DoubleRow
    ifmap AP is 3D).
  - ScalarE fuses the h_proj bias add + 1/64 descale + tanh in one pass,
    writing bf16; the v-dot is column-tiled M=1 matmuls as two concurrent
    groups of 4; ScalarE exp uses accum_out for fused softmax partials.
  - chunks prefetch 4 deep; chunk transposes ride the SP HWDGE queue.

Startup/teardown pipelining (preamble+tail used to idle the PE ~38us):

  - chunk (0,0) + We halves load first; chunk 0 contracts in two k-half
    passes across all 8 PSUM banks so pass A starts after half A of We.
  - chunk (0,0)/(0,1) run "deferred": raw pre-tanh e_proj is stored bf16
    and tanh'd at chunk (0,2), so nothing waits on Wh/h_proj early; Wh
    streams from inside the main loop. h_proj m-tiles + the 16 deferred
    tanh/v-dots interleave into chunk (0,2)'s stream.
  - hidden^T/b^T/v^T come from small padded xbar transposes on the ACT
    HWDGE queue instead of 4-byte gather DMAs.
  - ~40 throwaway matmuls on zeros warm the PE p-state during the
    preamble DMAs.
  - the last chunk's v-dot accumulates into one PSUM row (sequential
    M=1 matmuls interleaved with the m-groups), so the tail is just
    exp-from-PSUM -> reciprocal -> scale -> DMA.
"""
import numpy as np

B, S, H = 32, 2048, 1024
N_CORES = 8
B_LOCAL = B // N_CORES          # 4 batches per core
SL = B_LOCAL * S                # 8192 encoder rows per core
KT = H // 128                   # 8 contraction tiles
MT = H // 128                   # 8 output-H tiles
S_CHUNK = 512
RT = S_CHUNK // 128             # 4 row sub-tiles per chunk
N_CHUNKS = S // S_CHUNK         # 4 chunks per batch

_CACHE = {}


def _build(num_devices=N_CORES, reps=1):
    import concourse.mybir as mybir
    import concourse.tile as tile
    from concourse import bacc

    f32 = mybir.dt.float32

    nc = bacc.Bacc("TRN2", target_bir_lowering=False, debug=False,
                   num_devices=num_devices)
    enc = nc.dram_tensor("enc", [SL, H], f32, kind="ExternalInput").ap()
    hidden = nc.dram_tensor("hidden", [B_LOCAL, H], f32, kind="ExternalInput").ap()
    w_attn = nc.dram_tensor("w_attn", [2 * H, H], f32, kind="ExternalInput").ap()
    b_attn = nc.dram_tensor("b_attn", [H], f32, kind="ExternalInput").ap()
    v_in = nc.dram_tensor("v", [H], f32, kind="ExternalInput").ap()
    out = nc.dram_tensor("out", [B_LOCAL, S], f32, kind="ExternalOutput").ap()

    with tile.TileContext(nc) as tc:
        _emit(nc, tc, enc, hidden, w_attn, b_attn, v_in, out, reps=reps)

    nc.compile()
    return nc


def _emit(nc, tc, enc, hidden, w_attn, b_attn, v_in, out, reps=1):
    import concourse.mybir as mybir

    f32 = mybir.dt.float32
    bf16 = mybir.dt.bfloat16
    with (
        tc.tile_pool(name="weights", bufs=1) as w_pool,
        tc.tile_pool(name="small", bufs=1) as small_pool,
        tc.tile_pool(name="raw", bufs=4) as raw_pool,
        tc.tile_pool(name="xt", bufs=5) as xt_pool,
        tc.tile_pool(name="tanh", bufs=16) as tanh_pool,
        tc.tile_pool(name="xt8", bufs=4) as xt8_pool,
        tc.tile_pool(name="perbatch", bufs=3) as pb_pool,
        tc.tile_pool(name="psum_e", bufs=6, space="PSUM") as psum_e_pool,
        tc.tile_pool(name="psum_l", bufs=1, space="PSUM") as psum_l_pool,
        tc.tile_pool(name="psum_hp", bufs=1, space="PSUM") as psum_hp_pool,
    ):
        # All preamble loads ride the one SWDGE (Pool) queue in priority
        # order — its desc-gen pipeline issues transfers in exactly this
        # order, which the global DMA fabric then respects:
        #   chunk0 bands -> We m-blocks (cast bf16) -> chunks 1/2 -> smalls
        #   -> Wh. Transposes ride the otherwise-empty SP HWDGE queue.
        # we_sb/wh_sb layout [p, k, h_out]: h_in = k*128 + p (native rows).
        we_sb = w_pool.tile([128, KT, H], bf16)
        wh_sb = w_pool.tile([128, KT, H], bf16)

        # ---- chunk (0,0) raw load (4 row bands), then We halves (cast-DMA
        # to bf16). Chunk 0 contracts in two k-half passes, so pass A only
        # needs half A of We + this chunk's transpose. ----
        raw0 = raw_pool.tile([128, RT, H], bf16, tag="raw")
        nc.gpsimd.dma_start(
            out=raw0[:],
            in_=enc[0:S_CHUNK, :].rearrange("(r j) h -> j r h", j=128))
        nc.gpsimd.dma_start(
            out=we_sb[:, :4], in_=w_attn[H:H + 512].rearrange(
                "(k p) h -> p k h", p=128))
        nc.gpsimd.dma_start(
            out=we_sb[:, 4:], in_=w_attn[H + 512:].rearrange(
                "(k p) h -> p k h", p=128))
        # ---- small pads: tiny, loaded right behind We so their ACT-queue
        # transposes land long before chunk 2 needs them ----
        hidp = small_pool.tile([16, H], bf16)
        btp = small_pool.tile([16, 128], bf16)
        vtp = small_pool.tile([16, 128], bf16)
        nc.vector.memset(hidp[:], 0)
        nc.vector.memset(btp[:], 0)
        nc.vector.memset(vtp[:], 0)
        nc.gpsimd.dma_start(out=hidp[0:B_LOCAL, :], in_=hidden)
        nc.gpsimd.dma_start(out=btp[0:KT, :],
                            in_=b_attn.rearrange("(t p) -> t p", p=128))
        nc.gpsimd.dma_start(out=vtp[0:KT, :],
                            in_=v_in.rearrange("(t p) -> t p", p=128))
        # ---- scale We by 64 in place (exact in bf16; tanh descales by
        # 1/64) so the k0-1 fp8 pair tile is well-conditioned in e4m3;
        # per-(half, m-block) so pass A m-group m only waits half A ----
        for half in range(2):
            ks = slice(0, 4) if half == 0 else slice(4, KT)
            for m in range(MT):
                nc.vector.tensor_scalar_mul(
                    out=we_sb[:, ks, m * 128:(m + 1) * 128],
                    in0=we_sb[:, ks, m * 128:(m + 1) * 128], scalar1=64.0)
        # fp8 pair tile for the DoubleRow matmul over h_in 0:256
        we8 = w_pool.tile([128, 2, H], mybir.dt.float8e4)
        nc.vector.tensor_scalar_add(out=we8[:], in0=we_sb[:, 0:2, :],
                                    scalar1=0.0)
        # ---- chunks (0,1) and (0,2) raw prefetch ----
        raw1 = raw_pool.tile([128, RT, H], bf16, tag="raw")
        nc.gpsimd.dma_start(
            out=raw1[:],
            in_=enc[S_CHUNK:2 * S_CHUNK, :].rearrange("(r j) h -> j r h", j=128))
        raw2 = raw_pool.tile([128, RT, H], bf16, tag="raw")
        nc.gpsimd.dma_start(
            out=raw2[:],
            in_=enc[2 * S_CHUNK:3 * S_CHUNK, :].rearrange(
                "(r j) h -> j r h", j=128))

        # ---- transposes on the SP HWDGE queue (nothing else there) ----
        xt0 = xt_pool.tile([128, RT, KT, 128], bf16, tag="xt")
        nc.sync.dma_start_transpose(xt0[:], raw0[:])
        xt1 = xt_pool.tile([128, RT, KT, 128], bf16, tag="xt")
        nc.sync.dma_start_transpose(xt1[:], raw1[:])
        xt2 = xt_pool.tile([128, RT, KT, 128], bf16, tag="xt")
        nc.sync.dma_start_transpose(xt2[:], raw2[:])
        # ---- small transposes on the ACT HWDGE queue (its compute is
        # idle until chunk 0's PSUM copies) ----
        # ht[p, k, j] = hidp[j, k*128+p]; valid j < B_LOCAL
        ht = small_pool.tile([128, KT, 16], bf16)
        nc.scalar.dma_start_transpose(ht[:], hidp[:])
        btT = small_pool.tile([128, 16], bf16)   # btT[p, t] = b_attn[t*128+p]
        nc.scalar.dma_start_transpose(btT[:], btp[:])
        btT32 = small_pool.tile([128, 16], f32)
        nc.vector.tensor_scalar_add(out=btT32[:], in0=btT[:], scalar1=0.0)
        vtT = small_pool.tile([128, 16], bf16)   # vtT[p, t] = v[t*128+p]
        nc.scalar.dma_start_transpose(vtT[:], vtp[:])

        hp_sb = small_pool.tile([128, MT, B_LOCAL], f32)

        # ---- PE warm-up: throwaway matmuls on zeros while the chunk-0
        # DMAs land, so the real stream starts at full p-state ----
        warm = small_pool.tile([128, S_CHUNK], bf16)
        nc.vector.memset(warm[:], 0)
        ps_w = psum_l_pool.tile([128, S_CHUNK], f32, tag="psl")
        for _ in range(40):
            nc.tensor.matmul(ps_w[:], warm[:, 0:128], warm[:],
                             start=True, stop=True)

        pre = {"wh_sb": wh_sb, "ht": ht, "btT": btT32, "hp_sb": hp_sb,
               "psum_hp": psum_hp_pool, "w_attn": w_attn}
        fp8 = {"we8": we8, "xt8_pool": xt8_pool}

        # ---- main loop over (batch, s-chunk) ----
        for _rep in range(reps):
            _main_loop(nc, tc, mybir, enc, out, raw_pool, xt_pool, tanh_pool,
                       pb_pool, psum_e_pool, psum_l_pool, we_sb, vtT, hp_sb,
                       prefetched=({0: xt0, 1: xt1, 2: xt2}
                                   if _rep == 0 else None),
                       pre=(pre if _rep == 0 else None), fp8=fp8)


def _load_chunk(nc, mybir, enc, base, raw_pool, xt_pool):
    """Load 512 encoder rows (cast fp32->bf16), then one whole-chunk xbar
    transpose puts H on partitions:
      raw[j, r, h] = enc[base + r*128 + j, h]
      xt[p, r, k, j] = raw[j, r, k*128 + p]   (128-column block transpose)
    so xt[:, :, k, :] is a (strided) [128, 512] rhs tile."""
    bf16 = mybir.dt.bfloat16
    raw = raw_pool.tile([128, RT, H], bf16, tag="raw")
    nc.gpsimd.dma_start(
        out=raw[:],
        in_=enc[base:base + S_CHUNK, :].rearrange("(r j) h -> j r h", j=128))
    xt = xt_pool.tile([128, RT, KT, 128], bf16, tag="xt")
    nc.sync.dma_start_transpose(xt[:], raw[:])
    return xt


def _main_loop(nc, tc, mybir, enc, out, raw_pool, xt_pool, tanh_pool, pb_pool,
               psum_e_pool, psum_l_pool, we_sb, vt_sb, hp_sb, prefetched=None,
               pre=None, fp8=None):
    """Software-pipelined chunk loop. The second v-dot group of chunk i
    depends on that chunk's last tanh (ScalarE), so it is deferred into
    chunk i+1's PE stream (after its m=0 matmuls) — by then the tanh has
    long finished and the PE never stalls on ScalarE.

    When `pre` is given (first rep), chunk (0,0) additionally interleaves
    the weight-block casts and the h_proj m-tiles so the whole weight
    preamble overlaps the first chunk's compute."""
    f32 = mybir.dt.float32
    bf16 = mybir.dt.bfloat16

    def flush_pending(p):
        """Emit deferred v-dot group B + row-reduce + exp (+ batch tail)."""
        psl4 = p["psl4"]
        for j in range(4):
            m = j + 4
            nc.tensor.matmul(psl4[32 * j:32 * j + 1, :],
                             vt_sb[:, m:m + 1], p["ths"][m][:],
                             start=False, stop=True,
                             tile_position=(0, 32 * j))
        # reduce the 4 rows (ACT copies + DVE adds; 1 PSUM input max)
        s0 = pb_pool.tile([1, S_CHUNK], f32, tag="s0", bufs=2)
        s1 = pb_pool.tile([1, S_CHUNK], f32, tag="s1", bufs=2)
        t0 = pb_pool.tile([1, S_CHUNK], f32, tag="t0", bufs=2)
        t1 = pb_pool.tile([1, S_CHUNK], f32, tag="t1", bufs=2)
        od = pb_pool.tile([1, S_CHUNK], f32, tag="od", bufs=2)
        nc.scalar.copy(out=s0[:], in_=psl4[0:1, :])
        nc.vector.tensor_add(t0[:], psl4[32:33, :], s0[:])
        nc.scalar.copy(out=s1[:], in_=psl4[64:65, :])
        nc.vector.tensor_add(t1[:], psl4[96:97, :], s1[:])
        nc.vector.tensor_add(od[:], t0[:], t1[:])
        c, ex_sb, ssum = p["c"], p["ex_sb"], p["ssum"]
        nc.scalar.activation(
            out=ex_sb[0:1, c * S_CHUNK:(c + 1) * S_CHUNK],
            in_=od[:], func=mybir.ActivationFunctionType.Exp,
            accum_out=ssum[0:1, c:c + 1])
        if c == N_CHUNKS - 1:
            _batch_tail(p["b"], ex_sb, ssum)

    def _batch_tail(bb, ex_sb, ssum):
        # softmax normalize for the finished batch
        # (no max-subtraction needed: |logit| <= ~26)
        sm = pb_pool.tile([1, 2], f32, tag="sm")
        nc.vector.reduce_sum(out=sm[0:1, 0:1], in_=ssum[0:1, 0:N_CHUNKS],
                             axis=mybir.AxisListType.X)
        nc.vector.reciprocal(out=sm[0:1, 1:2], in_=sm[0:1, 0:1])
        ot = pb_pool.tile([1, S], f32, tag="ot", bufs=1)
        nc.vector.tensor_scalar_mul(out=ot[:], in0=ex_sb[:],
                                    scalar1=sm[0:1, 1:2])
        nc.sync.dma_start(out=out[bb:bb + 1, :], in_=ot[:])

    pending = None
    defer = None          # chunks (0,0)/(0,1) stored pre-tanh (first rep)
    dvd = None            # their deferred v-dot state during chunk (0,2)
    ex_sb = ssum = None
    # explicit 3-deep chunk prefetch so raw-DMA/transpose chains pipeline
    # instead of lockstepping with the PE
    n_seq = B_LOCAL * N_CHUNKS
    PF = 4
    xts = {}
    for i in range(PF):
        if prefetched is not None and i in prefetched:
            xts[i] = prefetched[i]
        else:
            xts[i] = _load_chunk(nc, mybir, enc, i * S_CHUNK, raw_pool,
                                 xt_pool)
    for b in range(B_LOCAL):
        ex_sb = pb_pool.tile([1, S], f32, tag="ex", bufs=2)
        ssum = pb_pool.tile([1, N_CHUNKS + 2], f32, tag="ssum")
        for c in range(N_CHUNKS):
            base = b * S + c * S_CHUNK
            last = (b == B_LOCAL - 1 and c == N_CHUNKS - 1)
            i_seq = b * N_CHUNKS + c
            if i_seq + PF < n_seq:
                xts[i_seq + PF] = _load_chunk(
                    nc, mybir, enc, (i_seq + PF) * S_CHUNK, raw_pool,
                    xt_pool)
            if pre is not None and i_seq == 0:
                # Wh loads here: after the early chunk raws, before h_proj
                # needs it at chunk (0,2)
                H_ = H
                nc.gpsimd.dma_start(
                    out=pre["wh_sb"][:, :4],
                    in_=pre["w_attn"][0:512].rearrange(
                        "(k p) h -> p k h", p=128))
                nc.gpsimd.dma_start(
                    out=pre["wh_sb"][:, 4:],
                    in_=pre["w_attn"][512:H_].rearrange(
                        "(k p) h -> p k h", p=128))
            xt = xts.pop(i_seq)
            # fp8 pair tile of this chunk's k-tiles 0..1 (pair dim
            # outermost so the lowered DoubleRow ifmap AP is 3D)
            xt8 = fp8["xt8_pool"].tile([128, 2, RT, 128],
                                       mybir.dt.float8e4, tag="xt8")
            nc.vector.tensor_scalar_add(
                out=xt8[:],
                in0=xt[:, :, 0:2, :].rearrange("p r k j -> p k r j"),
                scalar1=0.0)

            if pre is not None and b == 0 and c < 2:
                # --- deferred mode: h_proj isn't ready yet (Wh is still
                # streaming). Store raw pre-tanh e_proj tiles; tanh + v-dot
                # happen at chunk (0,2) once hp_sb exists. ---
                if defer is None:
                    defer = []
                if c == 0:
                    # two k-half passes across all 8 PSUM banks: pass A
                    # starts as soon as We half A + the transpose land
                    pses = []
                    for m in range(MT):
                        if m < 6:
                            pse = psum_e_pool.tile([128, S_CHUNK], f32)
                        elif m == 6:
                            pse = psum_l_pool.tile([128, S_CHUNK], f32,
                                                   tag="psl")
                        else:
                            pse = pre["psum_hp"].tile([128, S_CHUNK], f32,
                                                      tag="pshp")
                        pses.append(pse)
                    for m in range(MT):
                        nc.tensor.matmul(
                            pses[m][:],
                            fp8["we8"][:, :, m * 128:(m + 1) * 128],
                            xt8[:],
                            start=True, stop=False,
                            perf_mode=mybir.MatmulPerfMode.DoubleRow)
                        for k in range(2, KT // 2):
                            nc.tensor.matmul(
                                pses[m][:],
                                we_sb[:, k, m * 128:(m + 1) * 128],
                                xt[:, :, k, :],
                                start=False, stop=False)
                    for m in range(MT):
                        for k in range(KT // 2, KT):
                            nc.tensor.matmul(
                                pses[m][:],
                                we_sb[:, k, m * 128:(m + 1) * 128],
                                xt[:, :, k, :],
                                start=False, stop=(k == KT - 1))
                        est = tanh_pool.tile([128, S_CHUNK], bf16, tag="est")
                        nc.scalar.copy(out=est[:], in_=pses[m][:])
                        defer.append(est)
                else:
                    for m in range(MT):
                        pse = psum_e_pool.tile([128, S_CHUNK], f32)
                        nc.tensor.matmul(
                            pse[:],
                            fp8["we8"][:, :, m * 128:(m + 1) * 128],
                            xt8[:],
                            start=True, stop=False,
                            perf_mode=mybir.MatmulPerfMode.DoubleRow)
                        for k in range(2, KT):
                            nc.tensor.matmul(
                                pse[:],
                                we_sb[:, k, m * 128:(m + 1) * 128],
                                xt[:, :, k, :],
                                start=False, stop=(k == KT - 1))
                        est = tanh_pool.tile([128, S_CHUNK], bf16, tag="est")
                        nc.scalar.copy(out=est[:], in_=pse[:])
                        defer.append(est)
                continue

            if pre is not None and b == 0 and c == 2:
                # --- h_proj: 8 tiny m-tiles + bias adds (Wh landed during
                # chunks 0/1), then tanh the 16 stored tiles ---
                ps_hp = pre["psum_hp"].tile([128, MT * B_LOCAL], f32,
                                            tag="pshp")
                for m in range(MT):
                    for k in range(KT):
                        nc.tensor.matmul(
                            ps_hp[:, m * B_LOCAL:(m + 1) * B_LOCAL],
                            pre["wh_sb"][:, k, m * 128:(m + 1) * 128],
                            pre["ht"][:, k, 0:B_LOCAL],
                            start=(k == 0), stop=(k == KT - 1))
                for m in range(MT):
                    nc.vector.tensor_scalar_add(
                        out=hp_sb[:, m, :],
                        in0=ps_hp[:, m * B_LOCAL:(m + 1) * B_LOCAL],
                        scalar1=pre["btT"][:, m:m + 1])
                dths = []
                for i, est in enumerate(defer):
                    m = i % MT
                    dth = tanh_pool.tile([128, S_CHUNK], bf16)
                    nc.scalar.activation(
                        out=dth[:], in_=est[:],
                        func=mybir.ActivationFunctionType.Tanh,
                        bias=hp_sb[:, m, 0:1], scale=1.0 / 64.0)
                    dths.append(dth)
                # one PSUM bank, rows 0 / 32 for the two chunks' v-dots
                psl_d = pre["psum_hp"].tile([33, S_CHUNK], f32, tag="pshp")
                dvd = {"dths": dths, "psl_d": psl_d, "next": 0}
                defer = None

            psl4 = None
            psl_seq = None
            ths = []
            for m in range(MT):
                pse = psum_e_pool.tile([128, S_CHUNK], f32)
                nc.tensor.matmul(pse[:],
                                 fp8["we8"][:, :, m * 128:(m + 1) * 128],
                                 xt8[:],
                                 start=True, stop=False,
                                 perf_mode=mybir.MatmulPerfMode.DoubleRow)
                for k in range(2, KT):
                    nc.tensor.matmul(pse[:],
                                     we_sb[:, k, m * 128:(m + 1) * 128],
                                     xt[:, :, k, :],
                                     start=False, stop=(k == KT - 1))
                if last and m >= 1:
                    # sequential v-dot for m-1 (accumulate into one row)
                    nc.tensor.matmul(psl_seq[0:1, :],
                                     vt_sb[:, m - 1:m], ths[m - 1][:],
                                     start=(m == 1), stop=False)
                if dvd is not None and m >= 1:
                    # two deferred v-dots per m-group slot (16 total); the
                    # pair targets col groups 0/32 so it runs concurrently
                    for _ in range(2):
                        i = dvd["next"]
                        if i < 2 * MT:
                            mm, cc = divmod(i, 2)
                            nc.tensor.matmul(
                                dvd["psl_d"][32 * cc:32 * cc + 1, :],
                                vt_sb[:, mm:mm + 1],
                                dvd["dths"][cc * MT + mm][:],
                                start=(mm == 0), stop=(mm == MT - 1),
                                tile_position=(0, 32 * cc))
                            dvd["next"] = i + 1
                th = tanh_pool.tile([128, S_CHUNK], bf16)
                nc.scalar.activation(
                    out=th[:], in_=pse[:],
                    func=mybir.ActivationFunctionType.Tanh,
                    bias=hp_sb[:, m, b:b + 1], scale=1.0 / 64.0)
                ths.append(th)
                if m == 0:
                    if pending is not None:
                        flush_pending(pending)
                        pending = None
                    if last:
                        psl_seq = psum_l_pool.tile([128, S_CHUNK], f32,
                                                   tag="psl")
                if m == 4 and not last:
                    # v-dot group A (m=0..3): column-tiled, concurrent in
                    # the PE array; tanh m<=3 finished during the m=4 MMs
                    psl4 = psum_l_pool.tile([128, S_CHUNK], f32, tag="psl")
                    for j in range(4):
                        nc.tensor.matmul(psl4[32 * j:32 * j + 1, :],
                                         vt_sb[:, j:j + 1], ths[j][:],
                                         start=True, stop=False,
                                         tile_position=(0, 32 * j))
            if dvd is not None:
                # drain any remaining deferred v-dots, then their exps
                while dvd["next"] < 2 * MT:
                    i = dvd["next"]
                    mm, cc = divmod(i, 2)
                    nc.tensor.matmul(
                        dvd["psl_d"][32 * cc:32 * cc + 1, :],
                        vt_sb[:, mm:mm + 1],
                        dvd["dths"][cc * MT + mm][:],
                        start=(mm == 0), stop=(mm == MT - 1),
                        tile_position=(0, 32 * cc))
                    dvd["next"] = i + 1
                for cc in range(2):
                    nc.scalar.activation(
                        out=ex_sb[0:1, cc * S_CHUNK:(cc + 1) * S_CHUNK],
                        in_=dvd["psl_d"][32 * cc:32 * cc + 1, :],
                        func=mybir.ActivationFunctionType.Exp,
                        accum_out=ssum[0:1, cc:cc + 1])
                dvd = None
            if last:
                # final v-dot + tail: exp straight from PSUM, then normalize
                nc.tensor.matmul(psl_seq[0:1, :], vt_sb[:, MT - 1:MT],
                                 ths[MT - 1][:], start=False, stop=True)
                nc.scalar.activation(
                    out=ex_sb[0:1, c * S_CHUNK:(c + 1) * S_CHUNK],
                    in_=psl_seq[0:1, :],
                    func=mybir.ActivationFunctionType.Exp,
                    accum_out=ssum[0:1, c:c + 1])
                _batch_tail(b, ex_sb, ssum)
            else:
                pending = {"psl4": psl4, "ths": ths, "b": b, "c": c,
                           "ex_sb": ex_sb, "ssum": ssum}


def _build_runner():
    """Compile once and build a persistent jitted SPMD executor."""
    import jax
    from jax.sharding import Mesh, PartitionSpec
    from jax.experimental.shard_map import shard_map
    import concourse.mybir as mybir
    from concourse import bass2jax

    nc = _build()
    bass2jax.install_neuronx_cc_hook()

    partition_name = nc.partition_id_tensor.name if nc.partition_id_tensor else None
    in_names, out_names, out_avals, zero_outs = [], [], [], []
    for alloc in nc.m.functions[0].allocations:
        if not isinstance(alloc, mybir.MemoryLocationSet):
            continue
        name = alloc.memorylocations[0].name
        if alloc.kind == "ExternalInput":
            if name != partition_name:
                in_names.append(name)
        elif alloc.kind == "ExternalOutput":
            out_names.append(name)
            shape = tuple(alloc.tensor_shape)
            dtype = mybir.dt.np(alloc.dtype)
            out_avals.append(jax.core.ShapedArray(shape, dtype))
            zero_outs.append(np.zeros(shape, dtype))
    n_params = len(in_names)
    n_outs = len(out_avals)
    in_names = list(in_names) + list(out_names)
    if partition_name is not None:
        in_names.append(partition_name)
    donate = tuple(range(n_params, n_params + n_outs))

    def _body(*args):
        operands = list(args)
        if partition_name is not None:
            operands.append(bass2jax.partition_id_tensor())
        outs = bass2jax._bass_exec_p.bind(
            *operands,
            out_avals=tuple(out_avals),
            in_names=tuple(in_names),
            out_names=tuple(out_names),
            lowering_input_output_aliases=(),
            sim_require_finite=True,
            sim_require_nnan=True,
            nc=nc,
        )
        return tuple(outs)

    devices = jax.devices()[:N_CORES]
    assert len(devices) >= N_CORES, f"need {N_CORES} devices"
    mesh = Mesh(np.asarray(devices[:N_CORES]), ("core",))
    in_specs = (PartitionSpec("core"),) * (n_params + n_outs)
    out_specs = (PartitionSpec("core"),) * len(out_names)
    sharded = jax.jit(
        shard_map(_body, mesh=mesh, in_specs=in_specs, out_specs=out_specs,
                  check_rep=False),
        donate_argnums=donate, keep_unused=True)
    sharding = jax.sharding.NamedSharding(mesh, PartitionSpec("core"))

    state = {
        "sharded": sharded,
        "sharding": sharding,
        "in_names": in_names[:n_params],
        "out_names": out_names,
        "out_avals": out_avals,
        "zero_outs": zero_outs,
        "jax": jax,
    }
    return state


def _get_state():
    if "state" not in _CACHE:
        _CACHE["state"] = _build_runner()
    return _CACHE["state"]


def prepare_in_maps(hidden, encoder_outputs, W_attn, b_attn, v):
    """Shard inputs: batch-split encoder_outputs, replicate the rest."""
    enc = np.ascontiguousarray(np.asarray(encoder_outputs, dtype=np.float32))
    hid = np.ascontiguousarray(np.asarray(hidden, dtype=np.float32))
    W = np.ascontiguousarray(np.asarray(W_attn, dtype=np.float32))
    bb = np.ascontiguousarray(np.asarray(b_attn, dtype=np.float32))
    vv = np.ascontiguousarray(np.asarray(v, dtype=np.float32))
    in_maps = []
    for c in range(N_CORES):
        shard = enc[c * B_LOCAL:(c + 1) * B_LOCAL].reshape(SL, H)
        hshard = hid[c * B_LOCAL:(c + 1) * B_LOCAL]
        in_maps.append({"enc": shard, "hidden": hshard, "w_attn": W,
                        "b_attn": bb, "v": vv})
    return in_maps


def device_inputs(in_maps):
    st = _get_state()
    jax = st["jax"]
    concat_in = [
        np.concatenate([np.asarray(m[name]) for m in in_maps], axis=0)
        for name in st["in_names"]
    ]
    dev = [jax.device_put(a, st["sharding"]) for a in concat_in]
    jax.block_until_ready(dev)
    return dev


def run_device(dev_in):
    """One SPMD execution; returns the (B, S) fp32 output."""
    st = _get_state()
    jax = st["jax"]
    zeros = [
        jax.device_put(np.zeros((N_CORES * z.shape[0], *z.shape[1:]), z.dtype),
                       st["sharding"])
        for z in st["zero_outs"]
    ]
    out_arrs = st["sharded"](*dev_in, *zeros)
    jax.block_until_ready(out_arrs)
    i = st["out_names"].index("out")
    full = np.asarray(out_arrs[i]).reshape(N_CORES, B_LOCAL, S)
    return full.reshape(B, S)


def kernel(hidden, encoder_outputs, W_attn, b_attn, v):
    in_maps = prepare_in_maps(hidden, encoder_outputs, W_attn, b_attn, v)
    dev_in = device_inputs(in_maps)
    return run_device(dev_in).astype(np.float32)
